# revision 55
# baseline (speedup 1.0000x reference)
"""Trainium2 Bass kernel for nn_NetworkBasic (2-layer SLAYER SNN), v3.

Per core (batch sharded 2/core across 8 cores):
  stage A (TensorE): temporal matmul  mid = dataT^T @ T  (f16 hi/lo pair of
      the temporal matrix against exact 0/1 f16 data), evacuated to a
      w-PADDED f16 hi/lo mid pair (pad columns zero).
  stage B (TensorE): spatial 3x3 conv as banded-H f16 matmuls, 3 precision
      terms x 3 w-shifts per 4-step t-major block; the w-shifts use the
      padded mid so every matmul writes the full block; blocks are evacuated
      to the SBUF `what` buffer by ScalarE at full PE speed.
  scan (VectorE, ONE op/step): m[t+1] = (th>=m[t]) + 2d*m[t] + Wtil[t+1],
      where Wtil[t+1] = What[t+1] - d^2*m[t-1] is premerged IN PLACE in the
      what buffer by GpSimd (scalar_tensor_tensor, 2-step slack), so the
      serial VectorE chain is a single fused custom-DVE op.
  spikes: s = (m <= th) slabs on VectorE between scan steps; layer-1 slabs
      DMA out as produced.
  trans (TensorE): layer-0 spikes transposed per chunk into layer-1 dataT.

Membrane math: refractory alpha kernel ref[k] = A*k*d^k realized as an IIR
via scaled variables (c = 1/(A*d) < 0 flips >= to <=). FIR truncation tail
~1e-4 ignored.
"""

import os
import numpy as np

import concourse.bass as bass
import concourse.mybir as mybir
from concourse import bacc, bass_utils
from concourse.tile import TileContext
from concourse.masks import make_identity

F32 = mybir.dt.float32
F16 = mybir.dt.float16
AO = mybir.AluOpType

# ---------------- problem constants (hardcoded) ----------------
B_FULL, H, W, T = 16, 128, 64, 64
N_CORES = 8
B_LOC = B_FULL // N_CORES          # 2
BW = B_LOC * W                     # 128 (b,w) columns per core
SP_FREE = BW * T                   # 8192 free elements
WP = W + 2                         # padded w
MID_FREE = B_LOC * WP * T          # 8448
NPAIR = B_LOC * W // 2             # 64 transposed (b,w-pair) chunks
BLK = 8                            # stage-B t-block size
NBLK = T // BLK                    # 8 blocks

THETA = (30.0, 50.0)
TAU_SR = (1.0, 2.0)
TAU_REF = (1.0, 2.0)

PREMERGE = os.environ.get("KERNEL_PREMERGE", "gpsimd")  # gpsimd | vector
THR_ENGINE = os.environ.get("KERNEL_THR", "vector")     # vector | gpsimd


def _alpha_kernel(tau, mult, eps):
    vals = []
    for t in np.arange(0.0, float(T), 1.0):
        v = mult * t / tau * np.exp(1.0 - t / tau)
        if abs(v) < eps and t > tau:
            break
        vals.append(v)
    if len(vals) < 2:
        vals.append(0.0)
    return np.asarray(vals, np.float32)


SRM_K = [_alpha_kernel(TAU_SR[i], 1.0, 0.01) for i in range(2)]


def _layer_consts(layer):
    d = float(np.exp(-1.0 / TAU_REF[layer]))
    A = -2.0 * THETA[layer] * np.e / TAU_REF[layer]   # ref[k] = A*k*d^k
    c = 1.0 / (A * d)
    theta_hat = float(np.float32(c * THETA[layer]))
    return d, theta_hat


def _temporal_mat(layer):
    """[64,64] fp64 matrix:  what[t'] = sum_t data[t] * M[t, t']."""
    d, _ = _layer_consts(layer)
    A = -2.0 * THETA[layer] * np.e / TAU_REF[layer]
    c = 1.0 / (A * d)
    kern = SRM_K[layer].astype(np.float64)
    P = np.zeros((T, T))
    for t in range(T):
        for k in range(len(kern)):
            if t + k < T:
                P[t, t + k] = kern[k]
    D = np.zeros((T, T))
    for t in range(T):
        D[t, t] = 1.0
        if t + 1 < T:
            D[t, t + 1] = -2.0 * d
        if t + 2 < T:
            D[t, t + 2] = d * d
    return c * (P @ D)


def _hilo_f16(M):
    hi = M.astype(np.float16)
    lo = (M.astype(np.float32) - hi.astype(np.float32)).astype(np.float16)
    return hi, lo


def _hilo_f16_blockdiag(M):
    hi, lo = _hilo_f16(M)
    bhi = np.zeros((2 * T, 2 * T), np.float16)
    blo = np.zeros((2 * T, 2 * T), np.float16)
    for i in (0, 1):
        bhi[i * T:(i + 1) * T, i * T:(i + 1) * T] = hi
        blo[i * T:(i + 1) * T, i * T:(i + 1) * T] = lo
    return bhi, blo


def _h_mats(w, which):
    """w: [3,3] fp (pre-scaled) -> [3,128,128] f16; Hm[dwi][h,hp] = w[h-hp+1,dwi]."""
    out = np.zeros((3, H, H), np.float16)
    for dwi in range(3):
        for dh in (-1, 0, 1):
            v = np.float16(w[dh + 1, dwi]) if which == "hi" else np.float16(
                np.float32(w[dh + 1, dwi]) - np.float32(np.float16(w[dh + 1, dwi])))
            for hp in range(H):
                h = hp + dh
                if 0 <= h < H:
                    out[dwi, h, hp] = v
    return out


# ---------------- custom DVE op registration ----------------
_SNN_OPS = {}


def _register_snn(name, body_fn, ref_fn):
    if name in _SNN_OPS:
        return _SNN_OPS[name]
    import concourse.dve_ops as dve_ops
    from concourse.dve_spec import Spec, lower
    from concourse.dve_uop import DveOpSpec

    if name in dve_ops._SUB_OPCODE_FOR_NAME:
        op = next(op for op in dve_ops.OPS if op.name == name)
        _SNN_OPS[name] = op
        return op

    spec = Spec(body=body_fn(), reference=ref_fn)
    row = 1 + len(dve_ops.OPS)
    shas = {}
    for ver in ("v3", "v4"):
        try:
            tmp = DveOpSpec(name=name, opcode=row, uops=lower(spec, ver=ver), rd1_en=True)
            shas[ver] = tmp.sha(ver)
        except Exception:
            pass
    op = dve_ops.DveOp(name, spec, subdim=False, uops_sha=shas)
    dve_ops.OPS.append(op)
    dve_ops._SUB_OPCODE_FOR_NAME[name] = row
    dve_ops.CUSTOM_DVE_SPECS[name] = spec
    _SNN_OPS[name] = op
    return op


def _register_snn_op():
    # out = (s0 >= in0) + in0*s1 + in1
    from concourse.dve_spec import Src0, Src1, C0, C1
    return _register_snn(
        "SNN_STEP_ANT",
        lambda: (C0 >= Src0) + Src0 * C1 + Src1,
        lambda in0, in1, s0, s1, imm2: (
            (np.float32(s0) >= in0).astype(np.float32)
            + in0 * np.float32(s1) + in1
        ).astype(np.float32),
    )


def _register_snn_op2():
    # out = (s0 >= in0) + in0*s1 + in1*imm2
    from concourse.dve_spec import Src0, Src1, C0, C1, C2
    return _register_snn(
        "SNN_STEP2_ANT",
        lambda: (C0 >= Src0) + Src0 * C1 + Src1 * C2,
        lambda in0, in1, s0, s1, imm2: (
            (np.float32(s0) >= in0).astype(np.float32)
            + in0 * np.float32(s1) + in1 * np.float32(imm2)
        ).astype(np.float32),
    )


# ---------------- bass kernel trace ----------------
def trace_kernel(nc, xt_d, t_d, h_d, out_d):
    snn_op = _register_snn_op2()
    G = NPAIR // 4       # 16 stage-A groups of 4 chunks

    with TileContext(nc) as tc:
        with (
            tc.tile_pool(name="const", bufs=1) as cpool,
            tc.tile_pool(name="big", bufs=1) as bpool,
            tc.tile_pool(name="pa", bufs=2, space="PSUM") as pa_pool,
            tc.tile_pool(name="bq", bufs=2, space="PSUM") as bq_pool,
            tc.tile_pool(name="pt", bufs=2, space="PSUM") as pt_pool,
        ):
            # ---- constants + input, few big DMAs interleaved so stage A
            # can start after the first two issues ----
            ident = cpool.tile([H, H], F16)
            make_identity(nc, ident)
            dataT0 = bpool.tile([H, SP_FREE], F16, tag="dataT")
            tmats, hmats = {}, {}
            tmt = {}
            for layer in (0, 1):
                tm = cpool.tile([2 * T, 4 * T], F16, tag=f"t{layer}")
                tmt[layer] = tm
                tmats[layer] = (tm[:, :2 * T], tm[:, 2 * T:])
            nc.sync.dma_start(out=tmt[0], in_=t_d[0].ap())
            nc.sync.dma_start(out=dataT0[:, 0:2048], in_=xt_d.ap()[:, 0:2048])
            nc.sync.dma_start(out=dataT0[:, 2048:4096], in_=xt_d.ap()[:, 2048:4096])
            nc.sync.dma_start(out=dataT0[:, 4096:6144], in_=xt_d.ap()[:, 4096:6144])
            nc.sync.dma_start(out=dataT0[:, 6144:8192], in_=xt_d.ap()[:, 6144:8192])
            nc.sync.dma_start(out=tmt[1], in_=t_d[1].ap())
            for layer in (0, 1):
                hm = cpool.tile([H, 6 * H], F16, tag=f"h{layer}")
                nc.sync.dma_start(
                    out=hm[:, :].rearrange("p (s k n) -> p s k n", s=2, k=3),
                    in_=h_d[layer].ap().rearrange("s k p n -> p s k n"),
                )
                hmats[layer] = (hm[:, :3 * H], hm[:, 3 * H:])

            # padded mid pair; pad columns zeroed once (both layers reuse)
            midh = bpool.tile([H, MID_FREE], F16, tag="midh")
            midl = bpool.tile([H, MID_FREE], F16, tag="midl")
            for mtile in (midh, midl):
                for b in range(B_LOC):
                    nc.vector.memset(mtile[:, b * WP * T:b * WP * T + T], 0.0)
                    nc.vector.memset(
                        mtile[:, (b * WP + W + 1) * T:(b * WP + W + 2) * T], 0.0)

            dataT = dataT0
            for layer in (0, 1):
                d, theta_hat = _layer_consts(layer)
                two_d = float(np.float32(2.0 * d))
                md2 = float(np.float32(-(d * d)))
                thi, tlo = tmats[layer]
                hmh, hml = hmats[layer]

                # ---- stage A ----
                scopeA = nc.enter_named_scope(f"stageA{layer}", False)
                for g in range(G):
                    pa = pa_pool.tile([H, 4 * H], F32, tag="pa")
                    for c2 in range(4):
                        chunk = g * 4 + c2
                        lhsT = dataT[:, chunk * H:(chunk + 1) * H]
                        nc.tensor.matmul(
                            pa[:, c2 * H:(c2 + 1) * H], lhsT, thi,
                            start=True, stop=False, skip_group_check=True,
                        )
                        nc.tensor.matmul(
                            pa[:, c2 * H:(c2 + 1) * H], lhsT, tlo,
                            start=False, stop=True, skip_group_check=True,
                        )
                    b, w8 = divmod(g, G // B_LOC)
                    off = (b * WP + w8 * 8 + 1) * T
                    nc.scalar.copy(midh[:, off:off + 512], pa)
                    nc.vector.scalar_tensor_tensor(
                        midl[:, off:off + 512], midh[:, off:off + 512],
                        -1.0, pa, AO.mult, AO.add,
                    )
                nc.leave_named_scope(f"stageA{layer}", scopeA[0], False)

                # ---- stage B: f16 3-term, (b,w,t)-major blocks so the
                # moving operand's inner run is contiguous (16B bursts) ----
                what = bpool.tile([H, SP_FREE], F32, tag="what")
                whatv = what[:, :].rearrange("p (t b w) -> p t b w",
                                             t=T, b=B_LOC)
                mvh = midh[:, :].rearrange("p (b w t) -> p b w t", b=B_LOC, w=WP)
                mvl = midl[:, :].rearrange("p (b w t) -> p b w t", b=B_LOC, w=WP)
                scopeB = nc.enter_named_scope(f"stageB{layer}", False)
                for k in range(NBLK):
                    # one PSUM bank per b-half (2-bank tiles would round to 4)
                    bq0 = bq_pool.tile([H, BLK * W], F32, tag="bq0")
                    bq1 = bq_pool.tile([H, BLK * W], F32, tag="bq1")
                    bqs = (bq0, bq1)
                    ts = slice(k * BLK, (k + 1) * BLK)
                    first = True
                    for hm_, mv_ in ((hmh, mvh), (hmh, mvl), (hml, mvh)):
                        for dw in (0, -1, 1):
                            last = (hm_ is hml) and dw == 1
                            for b in range(B_LOC):
                                nc.tensor.matmul(
                                    bqs[b][:, :],
                                    hm_[:, (dw + 1) * H:(dw + 2) * H],
                                    mv_[:, b, 1 + dw:1 + dw + W, ts],
                                    start=first, stop=last,
                                    skip_group_check=True,
                                )
                            first = False
                    # strided evac: (w,t8) b-halves -> t-major what
                    for b in range(B_LOC):
                        src = bqs[b][:, :].rearrange("p (w t) -> p t w", t=BLK)
                        nc.scalar.copy(whatv[:, ts, b, :], src)
                nc.leave_named_scope(f"stageB{layer}", scopeB[0], False)

                # ---- scan: 1 fused DVE op/step + premerge on gpsimd ----
                mh = bpool.tile([H, SP_FREE], F32, tag="mh")
                spk = bpool.tile([H, SP_FREE], F16,
                                 tag="spk0" if layer == 0 else "spk1")
                spkv = spk[:, :].rearrange("p (b w t) -> p b w t",
                                           b=B_LOC, w=W)
                mhv = mh[:, :].rearrange("p (t b w) -> p b w t",
                                         t=T, b=B_LOC)

                def msl(t):
                    return mh[:, t * BW:(t + 1) * BW]

                def wsl(t):
                    return what[:, t * BW:(t + 1) * BW]

                prem = nc.gpsimd if PREMERGE == "gpsimd" else nc.vector

                # `what` holds Whatp = What/(-d^2) (H mats are host-scaled);
                # the DVE op multiplies in1 by imm2 = -d^2, so the premerge
                # is a PLAIN ADD (the only 2-tensor op GpSimd's ISA has):
                #   P[t+1]  = Whatp[t+1] + m[t-1]          (gpsimd, in place)
                #   m[t+1]  = (th>=m[t]) + 2d*m[t] + md2*P[t+1]   (DVE)
                scopeS = nc.enter_named_scope(f"scan{layer}", False)
                nc.vector.tensor_scalar(msl(0), wsl(0), md2, None, AO.mult)
                def vslot(s):
                    # slots premerged inline on vector; chosen so gpsimd is
                    # idle while the (SBUF-port-sharing) vector threshold
                    # slab runs right after steps 7/15/23/...
                    return s % 8 in (1, 2, 5)

                for t in range(T - 1):
                    # vector premerge for slot t+2, emitted BEFORE this
                    # step's DVE op so its drain is hidden by one op gap
                    s = t + 2
                    if 2 <= s <= T - 1 and vslot(s):
                        nc.vector.tensor_tensor(
                            wsl(s), msl(s - 2), wsl(s), AO.add)
                    # gpsimd premerge for slot t+1 (2-step slack)
                    s = t + 1
                    if s >= 2 and not vslot(s):
                        prem.tensor_tensor(
                            wsl(s), msl(s - 2), wsl(s), AO.add)
                    nc.vector._custom_dve(
                        snn_op, out=msl(t + 1), in0=msl(t),
                        in1=wsl(t + 1), s0=theta_hat, s1=two_d, imm2=md2,
                    )
                    # keep the PE HAM warm through scan0's tail (B0 is done
                    # by then) so trans1+A1 start at 2.4GHz, not 1.2
                    if layer == 0 and t >= 28 and t % 5 == 3:
                        pw = pt_pool.tile([H, 4 * H], F16, tag="ptr")
                        nc.tensor.transpose(
                            pw[:, :H], msl(t).bitcast(F16)[:, :H], ident)
                    # threshold finished 8-step slabs (small pieces so the
                    # vector queue never blocks for long)
                    if (t + 1) % 8 == 0 and (t + 1) < T:
                        t0s = t + 1 - 8
                        if layer == 0:
                            nc.vector.tensor_scalar(
                                spkv[:, :, :, t0s:t + 1],
                                mhv[:, :, :, t0s:t + 1],
                                theta_hat, None, AO.is_le)
                        else:
                            # contiguous slabs go on gpsimd, off the
                            # vector queue
                            sl = slice(t0s * BW, (t + 1) * BW)
                            nc.gpsimd.tensor_scalar(
                                spk[:, sl], mh[:, sl], theta_hat, None,
                                AO.is_le)
                            nc.sync.dma_start(
                                out=out_d.ap()[:, sl], in_=spk[:, sl])
                    # early final piece to shorten the tail
                    if layer == 1 and t == T - 3:
                        sl = slice((T - 8) * BW, (T - 2) * BW)
                        nc.gpsimd.tensor_scalar(
                            spk[:, sl], mh[:, sl], theta_hat, None,
                            AO.is_le)
                        nc.sync.dma_start(
                            out=out_d.ap()[:, sl], in_=spk[:, sl])
                nc.leave_named_scope(f"scan{layer}", scopeS[0], False)
                if layer == 0:
                    nc.vector.tensor_scalar(
                        spkv[:, :, :, T - 8:T], mhv[:, :, :, T - 8:T],
                        theta_hat, None, AO.is_le)
                else:
                    sl = slice((T - 2) * BW, T * BW)
                    nc.vector.tensor_scalar(
                        spk[:, sl], mh[:, sl], theta_hat, None, AO.is_le)
                    nc.sync.dma_start(out=out_d.ap()[:, sl], in_=spk[:, sl])

                if layer == 0:
                    # transpose s1 chunks on PE -> next layer's dataT
                    dataT2 = bpool.tile([H, SP_FREE], F16, tag="dataT2")
                    scopeT = nc.enter_named_scope("trans1", False)
                    for g in range(G):
                        ptr = pt_pool.tile([H, 4 * H], F16, tag="ptr")
                        for c2 in range(4):
                            chunk = g * 4 + c2
                            nc.tensor.transpose(
                                ptr[:, c2 * H:(c2 + 1) * H],
                                spk[:, chunk * H:(chunk + 1) * H], ident)
                        sl = slice(g * 512, (g + 1) * 512)
                        if g % 2 == 0:
                            nc.scalar.copy(dataT2[:, sl], ptr)
                        else:
                            nc.vector.tensor_copy(dataT2[:, sl], ptr)
                    nc.leave_named_scope("trans1", scopeT[0], False)
                    dataT = dataT2
    return nc


_BUILT = {}


def _build():
    global _BUILT
    key = (PREMERGE, THR_ENGINE)
    if key in _BUILT:
        return _BUILT[key]
    nc = bacc.Bacc("TRN2", debug=False)
    xt_d = nc.dram_tensor("xt", [H, SP_FREE], F16, kind="ExternalInput")
    t_d, h_d = {}, {}
    for layer in (0, 1):
        t_d[layer] = nc.dram_tensor(f"t{layer}", [2 * T, 4 * T], F16,
                                    kind="ExternalInput")
        h_d[layer] = nc.dram_tensor(f"h{layer}", [2, 3, H, H], F16,
                                    kind="ExternalInput")
    out_d = nc.dram_tensor("out", [H, SP_FREE], F16, kind="ExternalOutput")
    trace_kernel(nc, xt_d, t_d, h_d, out_d)
    nc.compile()
    _BUILT[key] = nc
    return nc


def _host_inputs(conv1_w, conv2_w):
    ins = {}
    for layer, w in ((0, conv1_w), (1, conv2_w)):
        hi, lo = _hilo_f16_blockdiag(_temporal_mat(layer))
        ins[f"t{layer}"] = np.hstack([hi, lo])
        d, _ = _layer_consts(layer)
        md2 = float(np.float32(-(d * d)))
        # stage B computes Whatp = What/(-d^2); the scan's DVE op multiplies
        # the premerged stream back by -d^2 (imm2)
        w2 = np.asarray(w, np.float32).reshape(3, 3) / np.float32(md2)
        ins[f"h{layer}"] = np.stack([_h_mats(w2, "hi"), _h_mats(w2, "lo")])
    return ins


def make_in_maps(spikeInput, conv1_w, conv2_w):
    x = np.asarray(spikeInput, np.float32).reshape(B_FULL, H, W, T)
    x16 = x.astype(np.float16)                      # exact: values are 0/1
    common = _host_inputs(conv1_w, conv2_w)
    in_maps = []
    for c in range(N_CORES):
        xc = x16[c * B_LOC:(c + 1) * B_LOC]         # [b, h, w, t]
        xc = xc.reshape(B_LOC, H, W // 2, 2, T)     # b h wp w2 t
        xt = np.ascontiguousarray(xc.transpose(3, 4, 0, 2, 1))  # w2 t b wp h
        m = dict(common)
        m["xt"] = xt.reshape(H, SP_FREE)
        in_maps.append(m)
    return in_maps


def kernel(spikeInput, conv1_w, conv2_w):
    nc = _build()
    in_maps = make_in_maps(spikeInput, conv1_w, conv2_w)
    res = bass_utils.run_bass_kernel_spmd(nc, in_maps, core_ids=list(range(N_CORES)))
    outs = []
    for r in res.results:
        o = r["out"].reshape(H, T, B_LOC, W)        # h t b w
        outs.append(o.transpose(2, 0, 3, 1))        # b h w t
    return np.concatenate(outs, axis=0).astype(np.float32)


# revision 56
# speedup vs baseline: 1.7428x; 1.7428x over previous
"""Trainium2 Bass kernel for nn_NetworkBasic (2-layer SLAYER SNN), v3.

Per core (batch sharded 2/core across 8 cores):
  stage A (TensorE): temporal matmul  mid = dataT^T @ T  (f16 hi/lo pair of
      the temporal matrix against exact 0/1 f16 data), evacuated to a
      w-PADDED f16 hi/lo mid pair (pad columns zero).
  stage B (TensorE): spatial 3x3 conv as banded-H f16 matmuls, 3 precision
      terms x 3 w-shifts per 4-step t-major block; the w-shifts use the
      padded mid so every matmul writes the full block; blocks are evacuated
      to the SBUF `what` buffer by ScalarE at full PE speed.
  scan (VectorE, ONE op/step): m[t+1] = (th>=m[t]) + 2d*m[t] + Wtil[t+1],
      where Wtil[t+1] = What[t+1] - d^2*m[t-1] is premerged IN PLACE in the
      what buffer by GpSimd (scalar_tensor_tensor, 2-step slack), so the
      serial VectorE chain is a single fused custom-DVE op.
  spikes: s = (m <= th) slabs on VectorE between scan steps; layer-1 slabs
      DMA out as produced.
  trans (TensorE): layer-0 spikes transposed per chunk into layer-1 dataT.

Membrane math: refractory alpha kernel ref[k] = A*k*d^k realized as an IIR
via scaled variables (c = 1/(A*d) < 0 flips >= to <=). FIR truncation tail
~1e-4 ignored.
"""

import os
import numpy as np

import concourse.bass as bass
import concourse.mybir as mybir
from concourse import bacc, bass_utils
from concourse.tile import TileContext
from concourse.masks import make_identity

F32 = mybir.dt.float32
F16 = mybir.dt.float16
AO = mybir.AluOpType

# ---------------- problem constants (hardcoded) ----------------
B_FULL, H, W, T = 16, 128, 64, 64
N_CORES = 8
B_LOC = B_FULL // N_CORES          # 2
BW = B_LOC * W                     # 128 (b,w) columns per core
SP_FREE = BW * T                   # 8192 free elements
WP = W + 2                         # padded w
MID_FREE = B_LOC * WP * T          # 8448
NPAIR = B_LOC * W // 2             # 64 transposed (b,w-pair) chunks
BLK = 8                            # stage-B t-block size
NBLK = T // BLK                    # 8 blocks

THETA = (30.0, 50.0)
TAU_SR = (1.0, 2.0)
TAU_REF = (1.0, 2.0)

PREMERGE = os.environ.get("KERNEL_PREMERGE", "gpsimd")  # gpsimd | vector
THR_ENGINE = os.environ.get("KERNEL_THR", "vector")     # vector | gpsimd


def _alpha_kernel(tau, mult, eps):
    vals = []
    for t in np.arange(0.0, float(T), 1.0):
        v = mult * t / tau * np.exp(1.0 - t / tau)
        if abs(v) < eps and t > tau:
            break
        vals.append(v)
    if len(vals) < 2:
        vals.append(0.0)
    return np.asarray(vals, np.float32)


SRM_K = [_alpha_kernel(TAU_SR[i], 1.0, 0.01) for i in range(2)]


def _layer_consts(layer):
    d = float(np.exp(-1.0 / TAU_REF[layer]))
    A = -2.0 * THETA[layer] * np.e / TAU_REF[layer]   # ref[k] = A*k*d^k
    c = 1.0 / (A * d)
    theta_hat = float(np.float32(c * THETA[layer]))
    return d, theta_hat


def _temporal_mat(layer):
    """[64,64] fp64 matrix:  what[t'] = sum_t data[t] * M[t, t']."""
    d, _ = _layer_consts(layer)
    A = -2.0 * THETA[layer] * np.e / TAU_REF[layer]
    c = 1.0 / (A * d)
    kern = SRM_K[layer].astype(np.float64)
    P = np.zeros((T, T))
    for t in range(T):
        for k in range(len(kern)):
            if t + k < T:
                P[t, t + k] = kern[k]
    D = np.zeros((T, T))
    for t in range(T):
        D[t, t] = 1.0
        if t + 1 < T:
            D[t, t + 1] = -2.0 * d
        if t + 2 < T:
            D[t, t + 2] = d * d
    return c * (P @ D)


def _hilo_f16(M):
    hi = M.astype(np.float16)
    lo = (M.astype(np.float32) - hi.astype(np.float32)).astype(np.float16)
    return hi, lo


def _hilo_f16_blockdiag(M):
    hi, lo = _hilo_f16(M)
    bhi = np.zeros((2 * T, 2 * T), np.float16)
    blo = np.zeros((2 * T, 2 * T), np.float16)
    for i in (0, 1):
        bhi[i * T:(i + 1) * T, i * T:(i + 1) * T] = hi
        blo[i * T:(i + 1) * T, i * T:(i + 1) * T] = lo
    return bhi, blo


def _h_mats(w, which):
    """w: [3,3] fp (pre-scaled) -> [3,128,128] f16; Hm[dwi][h,hp] = w[h-hp+1,dwi]."""
    out = np.zeros((3, H, H), np.float16)
    for dwi in range(3):
        for dh in (-1, 0, 1):
            v = np.float16(w[dh + 1, dwi]) if which == "hi" else np.float16(
                np.float32(w[dh + 1, dwi]) - np.float32(np.float16(w[dh + 1, dwi])))
            for hp in range(H):
                h = hp + dh
                if 0 <= h < H:
                    out[dwi, h, hp] = v
    return out


# ---------------- custom DVE op registration ----------------
_SNN_OPS = {}


def _register_snn(name, body_fn, ref_fn):
    if name in _SNN_OPS:
        return _SNN_OPS[name]
    import concourse.dve_ops as dve_ops
    from concourse.dve_spec import Spec, lower
    from concourse.dve_uop import DveOpSpec

    if name in dve_ops._SUB_OPCODE_FOR_NAME:
        op = next(op for op in dve_ops.OPS if op.name == name)
        _SNN_OPS[name] = op
        return op

    spec = Spec(body=body_fn(), reference=ref_fn)
    row = 1 + len(dve_ops.OPS)
    shas = {}
    for ver in ("v3", "v4"):
        try:
            tmp = DveOpSpec(name=name, opcode=row, uops=lower(spec, ver=ver), rd1_en=True)
            shas[ver] = tmp.sha(ver)
        except Exception:
            pass
    op = dve_ops.DveOp(name, spec, subdim=False, uops_sha=shas)
    dve_ops.OPS.append(op)
    dve_ops._SUB_OPCODE_FOR_NAME[name] = row
    dve_ops.CUSTOM_DVE_SPECS[name] = spec
    _SNN_OPS[name] = op
    return op


def _register_snn_op():
    # out = (s0 >= in0) + in0*s1 + in1
    from concourse.dve_spec import Src0, Src1, C0, C1
    return _register_snn(
        "SNN_STEP_ANT",
        lambda: (C0 >= Src0) + Src0 * C1 + Src1,
        lambda in0, in1, s0, s1, imm2: (
            (np.float32(s0) >= in0).astype(np.float32)
            + in0 * np.float32(s1) + in1
        ).astype(np.float32),
    )


def _register_snn_op2():
    # out = (s0 >= in0) + in0*s1 + in1*imm2
    from concourse.dve_spec import Src0, Src1, C0, C1, C2
    return _register_snn(
        "SNN_STEP2_ANT",
        lambda: (C0 >= Src0) + Src0 * C1 + Src1 * C2,
        lambda in0, in1, s0, s1, imm2: (
            (np.float32(s0) >= in0).astype(np.float32)
            + in0 * np.float32(s1) + in1 * np.float32(imm2)
        ).astype(np.float32),
    )


# ---------------- bass kernel trace ----------------
def trace_kernel(nc, xt_d, t_d, h_d, out_d):
    snn_op = _register_snn_op2()
    G = NPAIR // 4       # 16 stage-A groups of 4 chunks

    with TileContext(nc) as tc:
        with (
            tc.tile_pool(name="const", bufs=1) as cpool,
            tc.tile_pool(name="big", bufs=1) as bpool,
            tc.tile_pool(name="pa", bufs=2, space="PSUM") as pa_pool,
            tc.tile_pool(name="bq", bufs=2, space="PSUM") as bq_pool,
            tc.tile_pool(name="pt", bufs=2, space="PSUM") as pt_pool,
        ):
            # ---- constants + input, few big DMAs interleaved so stage A
            # can start after the first two issues ----
            ident = cpool.tile([H, H], F16)
            make_identity(nc, ident)
            dataT0 = bpool.tile([H, SP_FREE], F16, tag="dataT")
            tmats, hmats = {}, {}
            tmt = {}
            for layer in (0, 1):
                tm = cpool.tile([2 * T, 4 * T], F16, tag=f"t{layer}")
                tmt[layer] = tm
                tmats[layer] = (tm[:, :2 * T], tm[:, 2 * T:])
            nc.sync.dma_start(out=tmt[0], in_=t_d[0].ap())
            nc.sync.dma_start(out=dataT0[:, 0:2048], in_=xt_d.ap()[:, 0:2048])
            nc.sync.dma_start(out=dataT0[:, 2048:4096], in_=xt_d.ap()[:, 2048:4096])
            nc.sync.dma_start(out=dataT0[:, 4096:6144], in_=xt_d.ap()[:, 4096:6144])
            nc.sync.dma_start(out=dataT0[:, 6144:8192], in_=xt_d.ap()[:, 6144:8192])
            nc.sync.dma_start(out=tmt[1], in_=t_d[1].ap())
            for layer in (0, 1):
                hm = cpool.tile([H, 6 * H], F16, tag=f"h{layer}")
                nc.sync.dma_start(
                    out=hm[:, :].rearrange("p (s k n) -> p s k n", s=2, k=3),
                    in_=h_d[layer].ap().rearrange("s k p n -> p s k n"),
                )
                hmats[layer] = (hm[:, :3 * H], hm[:, 3 * H:])

            # padded mid pair; pad columns zeroed once (both layers reuse)
            midh = bpool.tile([H, MID_FREE], F16, tag="midh")
            midl = bpool.tile([H, MID_FREE], F16, tag="midl")
            for mtile in (midh, midl):
                for b in range(B_LOC):
                    nc.vector.memset(mtile[:, b * WP * T:b * WP * T + T], 0.0)
                    nc.vector.memset(
                        mtile[:, (b * WP + W + 1) * T:(b * WP + W + 2) * T], 0.0)

            dataT = dataT0
            for layer in (0, 1):
                d, theta_hat = _layer_consts(layer)
                two_d = float(np.float32(2.0 * d))
                md2 = float(np.float32(-(d * d)))
                thi, tlo = tmats[layer]
                hmh, hml = hmats[layer]

                # ---- stage A ----
                scopeA = nc.enter_named_scope(f"stageA{layer}", False)
                for g in range(G):
                    pa = pa_pool.tile([H, 4 * H], F32, tag="pa")
                    for c2 in range(4):
                        chunk = g * 4 + c2
                        lhsT = dataT[:, chunk * H:(chunk + 1) * H]
                        nc.tensor.matmul(
                            pa[:, c2 * H:(c2 + 1) * H], lhsT, thi,
                            start=True, stop=False, skip_group_check=True,
                        )
                        nc.tensor.matmul(
                            pa[:, c2 * H:(c2 + 1) * H], lhsT, tlo,
                            start=False, stop=True, skip_group_check=True,
                        )
                    b, w8 = divmod(g, G // B_LOC)
                    off = (b * WP + w8 * 8 + 1) * T
                    nc.scalar.copy(midh[:, off:off + 512], pa)
                    nc.vector.scalar_tensor_tensor(
                        midl[:, off:off + 512], midh[:, off:off + 512],
                        -1.0, pa, AO.mult, AO.add,
                    )
                nc.leave_named_scope(f"stageA{layer}", scopeA[0], False)

                # ---- stage B: f16 3-term, (b,w,t)-major blocks so the
                # moving operand's inner run is contiguous (16B bursts) ----
                what = bpool.tile([H, SP_FREE], F32, tag="what")
                whatv = what[:, :].rearrange("p (t b w) -> p t b w",
                                             t=T, b=B_LOC)
                mvh = midh[:, :].rearrange("p (b w t) -> p b w t", b=B_LOC, w=WP)
                mvl = midl[:, :].rearrange("p (b w t) -> p b w t", b=B_LOC, w=WP)
                scopeB = nc.enter_named_scope(f"stageB{layer}", False)
                for k in range(NBLK):
                    # one PSUM bank per b-half (2-bank tiles would round to 4)
                    bq0 = bq_pool.tile([H, BLK * W], F32, tag="bq0")
                    bq1 = bq_pool.tile([H, BLK * W], F32, tag="bq1")
                    bqs = (bq0, bq1)
                    ts = slice(k * BLK, (k + 1) * BLK)
                    first = True
                    for hm_, mv_ in ((hmh, mvh), (hmh, mvl), (hml, mvh)):
                        for dw in (0, -1, 1):
                            last = (hm_ is hml) and dw == 1
                            for b in range(B_LOC):
                                nc.tensor.matmul(
                                    bqs[b][:, :],
                                    hm_[:, (dw + 1) * H:(dw + 2) * H],
                                    mv_[:, b, 1 + dw:1 + dw + W, ts],
                                    start=first, stop=last,
                                    skip_group_check=True,
                                )
                            first = False
                    # strided evac: (w,t8) b-halves -> t-major what
                    for b in range(B_LOC):
                        src = bqs[b][:, :].rearrange("p (w t) -> p t w", t=BLK)
                        nc.scalar.copy(whatv[:, ts, b, :], src)
                nc.leave_named_scope(f"stageB{layer}", scopeB[0], False)

                # ---- scan: 1 fused DVE op/step + premerge on gpsimd ----
                mh = bpool.tile([H, SP_FREE], F32, tag="mh")
                spk = bpool.tile([H, SP_FREE], F16,
                                 tag="spk0" if layer == 0 else "spk1")
                spkv = spk[:, :].rearrange("p (b w t) -> p b w t",
                                           b=B_LOC, w=W)
                mhv = mh[:, :].rearrange("p (t b w) -> p b w t",
                                         t=T, b=B_LOC)

                def msl(t):
                    return mh[:, t * BW:(t + 1) * BW]

                def wsl(t):
                    return what[:, t * BW:(t + 1) * BW]

                prem = nc.gpsimd if PREMERGE == "gpsimd" else nc.vector

                # `what` holds Whatp = What/(-d^2) (H mats are host-scaled);
                # the DVE op multiplies in1 by imm2 = -d^2, so the premerge
                # is a PLAIN ADD (the only 2-tensor op GpSimd's ISA has):
                #   P[t+1]  = Whatp[t+1] + m[t-1]          (gpsimd, in place)
                #   m[t+1]  = (th>=m[t]) + 2d*m[t] + md2*P[t+1]   (DVE)
                scopeS = nc.enter_named_scope(f"scan{layer}", False)
                nc.vector.tensor_scalar(msl(0), wsl(0), md2, None, AO.mult)
                def vslot(s):
                    # slots premerged inline on vector; chosen so gpsimd is
                    # idle while the (SBUF-port-sharing) vector threshold
                    # slab runs right after steps 7/15/23/...
                    return s % 8 in (1, 2, 5)

                for t in range(T - 1):
                    # vector premerge for slot t+2, emitted BEFORE this
                    # step's DVE op so its drain is hidden by one op gap
                    s = t + 2
                    if 2 <= s <= T - 1 and vslot(s):
                        nc.vector.tensor_tensor(
                            wsl(s), msl(s - 2), wsl(s), AO.add)
                    # gpsimd premerge for slot t+1 (2-step slack)
                    s = t + 1
                    if s >= 2 and not vslot(s):
                        prem.tensor_tensor(
                            wsl(s), msl(s - 2), wsl(s), AO.add)
                    nc.vector._custom_dve(
                        snn_op, out=msl(t + 1), in0=msl(t),
                        in1=wsl(t + 1), s0=theta_hat, s1=two_d, imm2=md2,
                    )
                    # keep the PE HAM warm through scan0's tail (B0 is done
                    # by then) so trans1+A1 start at 2.4GHz, not 1.2
                    if layer == 0 and t >= 28 and t % 5 == 3:
                        pw = pt_pool.tile([H, 4 * H], F16, tag="ptr")
                        nc.tensor.transpose(
                            pw[:, :H], msl(t).bitcast(F16)[:, :H], ident)
                    # threshold finished 8-step slabs (small pieces so the
                    # vector queue never blocks for long)
                    if (t + 1) % 8 == 0 and (t + 1) < T:
                        t0s = t + 1 - 8
                        if layer == 0:
                            nc.vector.tensor_scalar(
                                spkv[:, :, :, t0s:t + 1],
                                mhv[:, :, :, t0s:t + 1],
                                theta_hat, None, AO.is_le)
                        else:
                            sl = slice(t0s * BW, (t + 1) * BW)
                            nc.vector.tensor_scalar(
                                spk[:, sl], mh[:, sl], theta_hat, None,
                                AO.is_le)
                            nc.sync.dma_start(
                                out=out_d.ap()[:, sl], in_=spk[:, sl])
                nc.leave_named_scope(f"scan{layer}", scopeS[0], False)
                if layer == 0:
                    nc.vector.tensor_scalar(
                        spkv[:, :, :, T - 8:T], mhv[:, :, :, T - 8:T],
                        theta_hat, None, AO.is_le)
                else:
                    sl = slice((T - 8) * BW, T * BW)
                    nc.vector.tensor_scalar(
                        spk[:, sl], mh[:, sl], theta_hat, None, AO.is_le)
                    nc.sync.dma_start(out=out_d.ap()[:, sl], in_=spk[:, sl])

                if layer == 0:
                    # transpose s1 chunks on PE -> next layer's dataT
                    dataT2 = bpool.tile([H, SP_FREE], F16, tag="dataT2")
                    scopeT = nc.enter_named_scope("trans1", False)
                    for g in range(G):
                        ptr = pt_pool.tile([H, 4 * H], F16, tag="ptr")
                        for c2 in range(4):
                            chunk = g * 4 + c2
                            nc.tensor.transpose(
                                ptr[:, c2 * H:(c2 + 1) * H],
                                spk[:, chunk * H:(chunk + 1) * H], ident)
                        sl = slice(g * 512, (g + 1) * 512)
                        if g % 2 == 0:
                            nc.scalar.copy(dataT2[:, sl], ptr)
                        else:
                            nc.vector.tensor_copy(dataT2[:, sl], ptr)
                    nc.leave_named_scope("trans1", scopeT[0], False)
                    dataT = dataT2
    return nc


_BUILT = {}


def _build():
    global _BUILT
    key = (PREMERGE, THR_ENGINE)
    if key in _BUILT:
        return _BUILT[key]
    nc = bacc.Bacc("TRN2", debug=False)
    xt_d = nc.dram_tensor("xt", [H, SP_FREE], F16, kind="ExternalInput")
    t_d, h_d = {}, {}
    for layer in (0, 1):
        t_d[layer] = nc.dram_tensor(f"t{layer}", [2 * T, 4 * T], F16,
                                    kind="ExternalInput")
        h_d[layer] = nc.dram_tensor(f"h{layer}", [2, 3, H, H], F16,
                                    kind="ExternalInput")
    out_d = nc.dram_tensor("out", [H, SP_FREE], F16, kind="ExternalOutput")
    trace_kernel(nc, xt_d, t_d, h_d, out_d)
    nc.compile()
    _BUILT[key] = nc
    return nc


def _host_inputs(conv1_w, conv2_w):
    ins = {}
    for layer, w in ((0, conv1_w), (1, conv2_w)):
        hi, lo = _hilo_f16_blockdiag(_temporal_mat(layer))
        ins[f"t{layer}"] = np.hstack([hi, lo])
        d, _ = _layer_consts(layer)
        md2 = float(np.float32(-(d * d)))
        # stage B computes Whatp = What/(-d^2); the scan's DVE op multiplies
        # the premerged stream back by -d^2 (imm2)
        w2 = np.asarray(w, np.float32).reshape(3, 3) / np.float32(md2)
        ins[f"h{layer}"] = np.stack([_h_mats(w2, "hi"), _h_mats(w2, "lo")])
    return ins


def make_in_maps(spikeInput, conv1_w, conv2_w):
    x = np.asarray(spikeInput, np.float32).reshape(B_FULL, H, W, T)
    x16 = x.astype(np.float16)                      # exact: values are 0/1
    common = _host_inputs(conv1_w, conv2_w)
    in_maps = []
    for c in range(N_CORES):
        xc = x16[c * B_LOC:(c + 1) * B_LOC]         # [b, h, w, t]
        xc = xc.reshape(B_LOC, H, W // 2, 2, T)     # b h wp w2 t
        xt = np.ascontiguousarray(xc.transpose(3, 4, 0, 2, 1))  # w2 t b wp h
        m = dict(common)
        m["xt"] = xt.reshape(H, SP_FREE)
        in_maps.append(m)
    return in_maps


def kernel(spikeInput, conv1_w, conv2_w):
    nc = _build()
    in_maps = make_in_maps(spikeInput, conv1_w, conv2_w)
    res = bass_utils.run_bass_kernel_spmd(nc, in_maps, core_ids=list(range(N_CORES)))
    outs = []
    for r in res.results:
        o = r["out"].reshape(H, T, B_LOC, W)        # h t b w
        outs.append(o.transpose(2, 0, 3, 1))        # b h w t
    return np.concatenate(outs, axis=0).astype(np.float32)


# revision 57
# speedup vs baseline: 1.7629x; 1.0115x over previous
"""Trainium2 Bass kernel for nn_NetworkBasic (2-layer SLAYER SNN), v3.

Per core (batch sharded 2/core across 8 cores):
  stage A (TensorE): temporal matmul  mid = dataT^T @ T  (f16 hi/lo pair of
      the temporal matrix against exact 0/1 f16 data), evacuated to a
      w-PADDED f16 hi/lo mid pair (pad columns zero).
  stage B (TensorE): spatial 3x3 conv as banded-H f16 matmuls, 3 precision
      terms x 3 w-shifts per 4-step t-major block; the w-shifts use the
      padded mid so every matmul writes the full block; blocks are evacuated
      to the SBUF `what` buffer by ScalarE at full PE speed.
  scan (VectorE, ONE op/step): m[t+1] = (th>=m[t]) + 2d*m[t] + Wtil[t+1],
      where Wtil[t+1] = What[t+1] - d^2*m[t-1] is premerged IN PLACE in the
      what buffer by GpSimd (scalar_tensor_tensor, 2-step slack), so the
      serial VectorE chain is a single fused custom-DVE op.
  spikes: s = (m <= th) slabs on VectorE between scan steps; layer-1 slabs
      DMA out as produced.
  trans (TensorE): layer-0 spikes transposed per chunk into layer-1 dataT.

Membrane math: refractory alpha kernel ref[k] = A*k*d^k realized as an IIR
via scaled variables (c = 1/(A*d) < 0 flips >= to <=). FIR truncation tail
~1e-4 ignored.
"""

import os
import numpy as np

import concourse.bass as bass
import concourse.mybir as mybir
from concourse import bacc, bass_utils
from concourse.tile import TileContext
from concourse.masks import make_identity

F32 = mybir.dt.float32
F16 = mybir.dt.float16
AO = mybir.AluOpType

# ---------------- problem constants (hardcoded) ----------------
B_FULL, H, W, T = 16, 128, 64, 64
N_CORES = 8
B_LOC = B_FULL // N_CORES          # 2
BW = B_LOC * W                     # 128 (b,w) columns per core
SP_FREE = BW * T                   # 8192 free elements
WP = W + 2                         # padded w
MID_FREE = B_LOC * WP * T          # 8448
NPAIR = B_LOC * W // 2             # 64 transposed (b,w-pair) chunks
BLK = 8                            # stage-B t-block size
NBLK = T // BLK                    # 8 blocks

THETA = (30.0, 50.0)
TAU_SR = (1.0, 2.0)
TAU_REF = (1.0, 2.0)

PREMERGE = os.environ.get("KERNEL_PREMERGE", "gpsimd")  # gpsimd | vector
THR_ENGINE = os.environ.get("KERNEL_THR", "vector")     # vector | gpsimd


def _alpha_kernel(tau, mult, eps):
    vals = []
    for t in np.arange(0.0, float(T), 1.0):
        v = mult * t / tau * np.exp(1.0 - t / tau)
        if abs(v) < eps and t > tau:
            break
        vals.append(v)
    if len(vals) < 2:
        vals.append(0.0)
    return np.asarray(vals, np.float32)


SRM_K = [_alpha_kernel(TAU_SR[i], 1.0, 0.01) for i in range(2)]


def _layer_consts(layer):
    d = float(np.exp(-1.0 / TAU_REF[layer]))
    A = -2.0 * THETA[layer] * np.e / TAU_REF[layer]   # ref[k] = A*k*d^k
    c = 1.0 / (A * d)
    theta_hat = float(np.float32(c * THETA[layer]))
    return d, theta_hat


def _temporal_mat(layer):
    """[64,64] fp64 matrix:  what[t'] = sum_t data[t] * M[t, t']."""
    d, _ = _layer_consts(layer)
    A = -2.0 * THETA[layer] * np.e / TAU_REF[layer]
    c = 1.0 / (A * d)
    kern = SRM_K[layer].astype(np.float64)
    P = np.zeros((T, T))
    for t in range(T):
        for k in range(len(kern)):
            if t + k < T:
                P[t, t + k] = kern[k]
    D = np.zeros((T, T))
    for t in range(T):
        D[t, t] = 1.0
        if t + 1 < T:
            D[t, t + 1] = -2.0 * d
        if t + 2 < T:
            D[t, t + 2] = d * d
    return c * (P @ D)


def _hilo_f16(M):
    hi = M.astype(np.float16)
    lo = (M.astype(np.float32) - hi.astype(np.float32)).astype(np.float16)
    return hi, lo


def _hilo_f16_blockdiag(M):
    hi, lo = _hilo_f16(M)
    bhi = np.zeros((2 * T, 2 * T), np.float16)
    blo = np.zeros((2 * T, 2 * T), np.float16)
    for i in (0, 1):
        bhi[i * T:(i + 1) * T, i * T:(i + 1) * T] = hi
        blo[i * T:(i + 1) * T, i * T:(i + 1) * T] = lo
    return bhi, blo


def _h_mats(w, which):
    """w: [3,3] fp (pre-scaled) -> [3,128,128] f16; Hm[dwi][h,hp] = w[h-hp+1,dwi]."""
    out = np.zeros((3, H, H), np.float16)
    for dwi in range(3):
        for dh in (-1, 0, 1):
            v = np.float16(w[dh + 1, dwi]) if which == "hi" else np.float16(
                np.float32(w[dh + 1, dwi]) - np.float32(np.float16(w[dh + 1, dwi])))
            for hp in range(H):
                h = hp + dh
                if 0 <= h < H:
                    out[dwi, h, hp] = v
    return out


# ---------------- custom DVE op registration ----------------
_SNN_OPS = {}


def _register_snn(name, body_fn, ref_fn):
    if name in _SNN_OPS:
        return _SNN_OPS[name]
    import concourse.dve_ops as dve_ops
    from concourse.dve_spec import Spec, lower
    from concourse.dve_uop import DveOpSpec

    if name in dve_ops._SUB_OPCODE_FOR_NAME:
        op = next(op for op in dve_ops.OPS if op.name == name)
        _SNN_OPS[name] = op
        return op

    spec = Spec(body=body_fn(), reference=ref_fn)
    row = 1 + len(dve_ops.OPS)
    shas = {}
    for ver in ("v3", "v4"):
        try:
            tmp = DveOpSpec(name=name, opcode=row, uops=lower(spec, ver=ver), rd1_en=True)
            shas[ver] = tmp.sha(ver)
        except Exception:
            pass
    op = dve_ops.DveOp(name, spec, subdim=False, uops_sha=shas)
    dve_ops.OPS.append(op)
    dve_ops._SUB_OPCODE_FOR_NAME[name] = row
    dve_ops.CUSTOM_DVE_SPECS[name] = spec
    _SNN_OPS[name] = op
    return op


def _register_snn_op():
    # out = (s0 >= in0) + in0*s1 + in1
    from concourse.dve_spec import Src0, Src1, C0, C1
    return _register_snn(
        "SNN_STEP_ANT",
        lambda: (C0 >= Src0) + Src0 * C1 + Src1,
        lambda in0, in1, s0, s1, imm2: (
            (np.float32(s0) >= in0).astype(np.float32)
            + in0 * np.float32(s1) + in1
        ).astype(np.float32),
    )


def _register_snn_op2():
    # out = (s0 >= in0) + in0*s1 + in1*imm2
    from concourse.dve_spec import Src0, Src1, C0, C1, C2
    return _register_snn(
        "SNN_STEP2_ANT",
        lambda: (C0 >= Src0) + Src0 * C1 + Src1 * C2,
        lambda in0, in1, s0, s1, imm2: (
            (np.float32(s0) >= in0).astype(np.float32)
            + in0 * np.float32(s1) + in1 * np.float32(imm2)
        ).astype(np.float32),
    )


# ---------------- bass kernel trace ----------------
def trace_kernel(nc, xt_d, t_d, h_d, out_d):
    snn_op = _register_snn_op2()
    G = NPAIR // 4       # 16 stage-A groups of 4 chunks

    with TileContext(nc) as tc:
        with (
            tc.tile_pool(name="const", bufs=1) as cpool,
            tc.tile_pool(name="big", bufs=1) as bpool,
            tc.tile_pool(name="pa", bufs=2, space="PSUM") as pa_pool,
            tc.tile_pool(name="bq", bufs=2, space="PSUM") as bq_pool,
            tc.tile_pool(name="pt", bufs=2, space="PSUM") as pt_pool,
        ):
            # ---- constants + input, few big DMAs interleaved so stage A
            # can start after the first two issues ----
            ident = cpool.tile([H, H], F16)
            make_identity(nc, ident)
            dataT0 = bpool.tile([H, SP_FREE], F16, tag="dataT")
            tmats, hmats = {}, {}
            tmt = {}
            for layer in (0, 1):
                tm = cpool.tile([2 * T, 4 * T], F16, tag=f"t{layer}")
                tmt[layer] = tm
                tmats[layer] = (tm[:, :2 * T], tm[:, 2 * T:])
            nc.sync.dma_start(out=dataT0[:, 0:2048], in_=xt_d.ap()[:, 0:2048])
            nc.sync.dma_start(out=tmt[0], in_=t_d[0].ap())
            nc.sync.dma_start(out=dataT0[:, 2048:4096], in_=xt_d.ap()[:, 2048:4096])
            nc.sync.dma_start(out=dataT0[:, 4096:6144], in_=xt_d.ap()[:, 4096:6144])
            nc.sync.dma_start(out=dataT0[:, 6144:8192], in_=xt_d.ap()[:, 6144:8192])
            nc.sync.dma_start(out=tmt[1], in_=t_d[1].ap())
            for layer in (0, 1):
                hm = cpool.tile([H, 6 * H], F16, tag=f"h{layer}")
                nc.sync.dma_start(
                    out=hm[:, :].rearrange("p (s k n) -> p s k n", s=2, k=3),
                    in_=h_d[layer].ap().rearrange("s k p n -> p s k n"),
                )
                hmats[layer] = (hm[:, :3 * H], hm[:, 3 * H:])

            # padded mid pair; pad columns zeroed once (both layers reuse)
            midh = bpool.tile([H, MID_FREE], F16, tag="midh")
            midl = bpool.tile([H, MID_FREE], F16, tag="midl")
            for mtile in (midh, midl):
                for b in range(B_LOC):
                    nc.vector.memset(mtile[:, b * WP * T:b * WP * T + T], 0.0)
                    nc.vector.memset(
                        mtile[:, (b * WP + W + 1) * T:(b * WP + W + 2) * T], 0.0)

            dataT = dataT0
            for layer in (0, 1):
                d, theta_hat = _layer_consts(layer)
                two_d = float(np.float32(2.0 * d))
                md2 = float(np.float32(-(d * d)))
                thi, tlo = tmats[layer]
                hmh, hml = hmats[layer]

                # ---- stage A ----
                scopeA = nc.enter_named_scope(f"stageA{layer}", False)
                for g in range(G):
                    pa = pa_pool.tile([H, 4 * H], F32, tag="pa")
                    for c2 in range(4):
                        chunk = g * 4 + c2
                        lhsT = dataT[:, chunk * H:(chunk + 1) * H]
                        nc.tensor.matmul(
                            pa[:, c2 * H:(c2 + 1) * H], lhsT, thi,
                            start=True, stop=False, skip_group_check=True,
                        )
                        nc.tensor.matmul(
                            pa[:, c2 * H:(c2 + 1) * H], lhsT, tlo,
                            start=False, stop=True, skip_group_check=True,
                        )
                    b, w8 = divmod(g, G // B_LOC)
                    off = (b * WP + w8 * 8 + 1) * T
                    nc.scalar.copy(midh[:, off:off + 512], pa)
                    nc.vector.scalar_tensor_tensor(
                        midl[:, off:off + 512], midh[:, off:off + 512],
                        -1.0, pa, AO.mult, AO.add,
                    )
                nc.leave_named_scope(f"stageA{layer}", scopeA[0], False)

                # ---- stage B: f16 3-term, (b,w,t)-major blocks so the
                # moving operand's inner run is contiguous (16B bursts) ----
                what = bpool.tile([H, SP_FREE], F32, tag="what")
                whatv = what[:, :].rearrange("p (t b w) -> p t b w",
                                             t=T, b=B_LOC)
                mvh = midh[:, :].rearrange("p (b w t) -> p b w t", b=B_LOC, w=WP)
                mvl = midl[:, :].rearrange("p (b w t) -> p b w t", b=B_LOC, w=WP)
                scopeB = nc.enter_named_scope(f"stageB{layer}", False)
                for k in range(NBLK):
                    # one PSUM bank per b-half (2-bank tiles would round to 4)
                    bq0 = bq_pool.tile([H, BLK * W], F32, tag="bq0")
                    bq1 = bq_pool.tile([H, BLK * W], F32, tag="bq1")
                    bqs = (bq0, bq1)
                    ts = slice(k * BLK, (k + 1) * BLK)
                    first = True
                    for hm_, mv_ in ((hmh, mvh), (hmh, mvl), (hml, mvh)):
                        for dw in (0, -1, 1):
                            last = (hm_ is hml) and dw == 1
                            for b in range(B_LOC):
                                nc.tensor.matmul(
                                    bqs[b][:, :],
                                    hm_[:, (dw + 1) * H:(dw + 2) * H],
                                    mv_[:, b, 1 + dw:1 + dw + W, ts],
                                    start=first, stop=last,
                                    skip_group_check=True,
                                )
                            first = False
                    # strided evac: (w,t8) b-halves -> t-major what
                    for b in range(B_LOC):
                        src = bqs[b][:, :].rearrange("p (w t) -> p t w", t=BLK)
                        nc.scalar.copy(whatv[:, ts, b, :], src)
                nc.leave_named_scope(f"stageB{layer}", scopeB[0], False)

                # ---- scan: 1 fused DVE op/step + premerge on gpsimd ----
                mh = bpool.tile([H, SP_FREE], F32, tag="mh")
                spk = bpool.tile([H, SP_FREE], F16,
                                 tag="spk0" if layer == 0 else "spk1")
                spkv = spk[:, :].rearrange("p (b w t) -> p b w t",
                                           b=B_LOC, w=W)
                mhv = mh[:, :].rearrange("p (t b w) -> p b w t",
                                         t=T, b=B_LOC)

                def msl(t):
                    return mh[:, t * BW:(t + 1) * BW]

                def wsl(t):
                    return what[:, t * BW:(t + 1) * BW]

                prem = nc.gpsimd if PREMERGE == "gpsimd" else nc.vector

                # `what` holds Whatp = What/(-d^2) (H mats are host-scaled);
                # the DVE op multiplies in1 by imm2 = -d^2, so the premerge
                # is a PLAIN ADD (the only 2-tensor op GpSimd's ISA has):
                #   P[t+1]  = Whatp[t+1] + m[t-1]          (gpsimd, in place)
                #   m[t+1]  = (th>=m[t]) + 2d*m[t] + md2*P[t+1]   (DVE)
                scopeS = nc.enter_named_scope(f"scan{layer}", False)
                nc.vector.tensor_scalar(msl(0), wsl(0), md2, None, AO.mult)
                def vslot(s):
                    # slots premerged inline on vector; chosen so gpsimd is
                    # idle while the (SBUF-port-sharing) vector threshold
                    # slab runs right after steps 7/15/23/...
                    return s % 8 in (1, 2, 5)

                for t in range(T - 1):
                    # vector premerge for slot t+2, emitted BEFORE this
                    # step's DVE op so its drain is hidden by one op gap
                    s = t + 2
                    if 2 <= s <= T - 1 and vslot(s):
                        nc.vector.tensor_tensor(
                            wsl(s), msl(s - 2), wsl(s), AO.add)
                    # gpsimd premerge for slot t+1 (2-step slack)
                    s = t + 1
                    if s >= 2 and not vslot(s):
                        prem.tensor_tensor(
                            wsl(s), msl(s - 2), wsl(s), AO.add)
                    nc.vector._custom_dve(
                        snn_op, out=msl(t + 1), in0=msl(t),
                        in1=wsl(t + 1), s0=theta_hat, s1=two_d, imm2=md2,
                    )
                    # early final piece so only 2 steps remain after the loop
                    if layer == 1 and t == T - 3:
                        sl = slice((T - 8) * BW, (T - 2) * BW)
                        nc.vector.tensor_scalar(
                            spk[:, sl], mh[:, sl], theta_hat, None,
                            AO.is_le)
                        nc.sync.dma_start(
                            out=out_d.ap()[:, sl], in_=spk[:, sl])
                    # keep the PE HAM warm through scan0's tail (B0 is done
                    # by then) so trans1+A1 start at 2.4GHz, not 1.2
                    if layer == 0 and t >= 24 and t % 4 == 1:
                        pw = pt_pool.tile([H, 4 * H], F16, tag="ptr")
                        nc.tensor.transpose(
                            pw[:, :H], msl(t).bitcast(F16)[:, :H], ident)
                    # threshold finished 8-step slabs (small pieces so the
                    # vector queue never blocks for long)
                    if (t + 1) % 8 == 0 and (t + 1) < T:
                        t0s = t + 1 - 8
                        if layer == 0:
                            nc.vector.tensor_scalar(
                                spkv[:, :, :, t0s:t + 1],
                                mhv[:, :, :, t0s:t + 1],
                                theta_hat, None, AO.is_le)
                        else:
                            sl = slice(t0s * BW, (t + 1) * BW)
                            nc.vector.tensor_scalar(
                                spk[:, sl], mh[:, sl], theta_hat, None,
                                AO.is_le)
                            nc.sync.dma_start(
                                out=out_d.ap()[:, sl], in_=spk[:, sl])
                nc.leave_named_scope(f"scan{layer}", scopeS[0], False)
                if layer == 0:
                    nc.vector.tensor_scalar(
                        spkv[:, :, :, T - 8:T], mhv[:, :, :, T - 8:T],
                        theta_hat, None, AO.is_le)
                else:
                    sl = slice((T - 2) * BW, T * BW)
                    nc.vector.tensor_scalar(
                        spk[:, sl], mh[:, sl], theta_hat, None, AO.is_le)
                    nc.sync.dma_start(out=out_d.ap()[:, sl], in_=spk[:, sl])

                if layer == 0:
                    # transpose s1 chunks on PE -> next layer's dataT
                    dataT2 = bpool.tile([H, SP_FREE], F16, tag="dataT2")
                    scopeT = nc.enter_named_scope("trans1", False)
                    for g in range(G):
                        ptr = pt_pool.tile([H, 4 * H], F16, tag="ptr")
                        for c2 in range(4):
                            chunk = g * 4 + c2
                            nc.tensor.transpose(
                                ptr[:, c2 * H:(c2 + 1) * H],
                                spk[:, chunk * H:(chunk + 1) * H], ident)
                        sl = slice(g * 512, (g + 1) * 512)
                        if g % 2 == 0:
                            nc.scalar.copy(dataT2[:, sl], ptr)
                        else:
                            nc.vector.tensor_copy(dataT2[:, sl], ptr)
                    nc.leave_named_scope("trans1", scopeT[0], False)
                    dataT = dataT2
    return nc


_BUILT = {}


def _build():
    global _BUILT
    key = (PREMERGE, THR_ENGINE)
    if key in _BUILT:
        return _BUILT[key]
    nc = bacc.Bacc("TRN2", debug=False)
    xt_d = nc.dram_tensor("xt", [H, SP_FREE], F16, kind="ExternalInput")
    t_d, h_d = {}, {}
    for layer in (0, 1):
        t_d[layer] = nc.dram_tensor(f"t{layer}", [2 * T, 4 * T], F16,
                                    kind="ExternalInput")
        h_d[layer] = nc.dram_tensor(f"h{layer}", [2, 3, H, H], F16,
                                    kind="ExternalInput")
    out_d = nc.dram_tensor("out", [H, SP_FREE], F16, kind="ExternalOutput")
    trace_kernel(nc, xt_d, t_d, h_d, out_d)
    nc.compile()
    _BUILT[key] = nc
    return nc


def _host_inputs(conv1_w, conv2_w):
    ins = {}
    for layer, w in ((0, conv1_w), (1, conv2_w)):
        hi, lo = _hilo_f16_blockdiag(_temporal_mat(layer))
        ins[f"t{layer}"] = np.hstack([hi, lo])
        d, _ = _layer_consts(layer)
        md2 = float(np.float32(-(d * d)))
        # stage B computes Whatp = What/(-d^2); the scan's DVE op multiplies
        # the premerged stream back by -d^2 (imm2)
        w2 = np.asarray(w, np.float32).reshape(3, 3) / np.float32(md2)
        ins[f"h{layer}"] = np.stack([_h_mats(w2, "hi"), _h_mats(w2, "lo")])
    return ins


def make_in_maps(spikeInput, conv1_w, conv2_w):
    x = np.asarray(spikeInput, np.float32).reshape(B_FULL, H, W, T)
    x16 = x.astype(np.float16)                      # exact: values are 0/1
    common = _host_inputs(conv1_w, conv2_w)
    in_maps = []
    for c in range(N_CORES):
        xc = x16[c * B_LOC:(c + 1) * B_LOC]         # [b, h, w, t]
        xc = xc.reshape(B_LOC, H, W // 2, 2, T)     # b h wp w2 t
        xt = np.ascontiguousarray(xc.transpose(3, 4, 0, 2, 1))  # w2 t b wp h
        m = dict(common)
        m["xt"] = xt.reshape(H, SP_FREE)
        in_maps.append(m)
    return in_maps


def kernel(spikeInput, conv1_w, conv2_w):
    nc = _build()
    in_maps = make_in_maps(spikeInput, conv1_w, conv2_w)
    res = bass_utils.run_bass_kernel_spmd(nc, in_maps, core_ids=list(range(N_CORES)))
    outs = []
    for r in res.results:
        o = r["out"].reshape(H, T, B_LOC, W)        # h t b w
        outs.append(o.transpose(2, 0, 3, 1))        # b h w t
    return np.concatenate(outs, axis=0).astype(np.float32)


# revision 58
# speedup vs baseline: 1.7758x; 1.0073x over previous
"""Trainium2 Bass kernel for nn_NetworkBasic (2-layer SLAYER SNN), v3.

Per core (batch sharded 2/core across 8 cores):
  stage A (TensorE): temporal matmul  mid = dataT^T @ T  (f16 hi/lo pair of
      the temporal matrix against exact 0/1 f16 data), evacuated to a
      w-PADDED f16 hi/lo mid pair (pad columns zero).
  stage B (TensorE): spatial 3x3 conv as banded-H f16 matmuls, 3 precision
      terms x 3 w-shifts per 4-step t-major block; the w-shifts use the
      padded mid so every matmul writes the full block; blocks are evacuated
      to the SBUF `what` buffer by ScalarE at full PE speed.
  scan (VectorE, ONE op/step): m[t+1] = (th>=m[t]) + 2d*m[t] + Wtil[t+1],
      where Wtil[t+1] = What[t+1] - d^2*m[t-1] is premerged IN PLACE in the
      what buffer by GpSimd (scalar_tensor_tensor, 2-step slack), so the
      serial VectorE chain is a single fused custom-DVE op.
  spikes: s = (m <= th) slabs on VectorE between scan steps; layer-1 slabs
      DMA out as produced.
  trans (TensorE): layer-0 spikes transposed per chunk into layer-1 dataT.

Membrane math: refractory alpha kernel ref[k] = A*k*d^k realized as an IIR
via scaled variables (c = 1/(A*d) < 0 flips >= to <=). FIR truncation tail
~1e-4 ignored.
"""

import os
import numpy as np

import concourse.bass as bass
import concourse.mybir as mybir
from concourse import bacc, bass_utils
from concourse.tile import TileContext
from concourse.masks import make_identity

F32 = mybir.dt.float32
F16 = mybir.dt.float16
AO = mybir.AluOpType

# ---------------- problem constants (hardcoded) ----------------
B_FULL, H, W, T = 16, 128, 64, 64
N_CORES = 8
B_LOC = B_FULL // N_CORES          # 2
BW = B_LOC * W                     # 128 (b,w) columns per core
SP_FREE = BW * T                   # 8192 free elements
WP = W + 2                         # padded w
MID_FREE = B_LOC * WP * T          # 8448
NPAIR = B_LOC * W // 2             # 64 transposed (b,w-pair) chunks
BLK = 8                            # stage-B t-block size
NBLK = T // BLK                    # 8 blocks

THETA = (30.0, 50.0)
TAU_SR = (1.0, 2.0)
TAU_REF = (1.0, 2.0)

PREMERGE = os.environ.get("KERNEL_PREMERGE", "gpsimd")  # gpsimd | vector
THR_ENGINE = os.environ.get("KERNEL_THR", "vector")     # vector | gpsimd


def _alpha_kernel(tau, mult, eps):
    vals = []
    for t in np.arange(0.0, float(T), 1.0):
        v = mult * t / tau * np.exp(1.0 - t / tau)
        if abs(v) < eps and t > tau:
            break
        vals.append(v)
    if len(vals) < 2:
        vals.append(0.0)
    return np.asarray(vals, np.float32)


SRM_K = [_alpha_kernel(TAU_SR[i], 1.0, 0.01) for i in range(2)]


def _layer_consts(layer):
    d = float(np.exp(-1.0 / TAU_REF[layer]))
    A = -2.0 * THETA[layer] * np.e / TAU_REF[layer]   # ref[k] = A*k*d^k
    c = 1.0 / (A * d)
    theta_hat = float(np.float32(c * THETA[layer]))
    return d, theta_hat


def _temporal_mat(layer):
    """[64,64] fp64 matrix:  what[t'] = sum_t data[t] * M[t, t']."""
    d, _ = _layer_consts(layer)
    A = -2.0 * THETA[layer] * np.e / TAU_REF[layer]
    c = 1.0 / (A * d)
    kern = SRM_K[layer].astype(np.float64)
    P = np.zeros((T, T))
    for t in range(T):
        for k in range(len(kern)):
            if t + k < T:
                P[t, t + k] = kern[k]
    D = np.zeros((T, T))
    for t in range(T):
        D[t, t] = 1.0
        if t + 1 < T:
            D[t, t + 1] = -2.0 * d
        if t + 2 < T:
            D[t, t + 2] = d * d
    return c * (P @ D)


def _hilo_f16(M):
    hi = M.astype(np.float16)
    lo = (M.astype(np.float32) - hi.astype(np.float32)).astype(np.float16)
    return hi, lo


def _hilo_f16_blockdiag(M):
    hi, lo = _hilo_f16(M)
    bhi = np.zeros((2 * T, 2 * T), np.float16)
    blo = np.zeros((2 * T, 2 * T), np.float16)
    for i in (0, 1):
        bhi[i * T:(i + 1) * T, i * T:(i + 1) * T] = hi
        blo[i * T:(i + 1) * T, i * T:(i + 1) * T] = lo
    return bhi, blo


def _h_mats(w, which):
    """w: [3,3] fp (pre-scaled) -> [3,128,128] f16; Hm[dwi][h,hp] = w[h-hp+1,dwi]."""
    out = np.zeros((3, H, H), np.float16)
    for dwi in range(3):
        for dh in (-1, 0, 1):
            v = np.float16(w[dh + 1, dwi]) if which == "hi" else np.float16(
                np.float32(w[dh + 1, dwi]) - np.float32(np.float16(w[dh + 1, dwi])))
            for hp in range(H):
                h = hp + dh
                if 0 <= h < H:
                    out[dwi, h, hp] = v
    return out


# ---------------- custom DVE op registration ----------------
_SNN_OPS = {}


def _register_snn(name, body_fn, ref_fn):
    if name in _SNN_OPS:
        return _SNN_OPS[name]
    import concourse.dve_ops as dve_ops
    from concourse.dve_spec import Spec, lower
    from concourse.dve_uop import DveOpSpec

    if name in dve_ops._SUB_OPCODE_FOR_NAME:
        op = next(op for op in dve_ops.OPS if op.name == name)
        _SNN_OPS[name] = op
        return op

    spec = Spec(body=body_fn(), reference=ref_fn)
    row = 1 + len(dve_ops.OPS)
    shas = {}
    for ver in ("v3", "v4"):
        try:
            tmp = DveOpSpec(name=name, opcode=row, uops=lower(spec, ver=ver), rd1_en=True)
            shas[ver] = tmp.sha(ver)
        except Exception:
            pass
    op = dve_ops.DveOp(name, spec, subdim=False, uops_sha=shas)
    dve_ops.OPS.append(op)
    dve_ops._SUB_OPCODE_FOR_NAME[name] = row
    dve_ops.CUSTOM_DVE_SPECS[name] = spec
    _SNN_OPS[name] = op
    return op


def _register_snn_op():
    # out = (s0 >= in0) + in0*s1 + in1
    from concourse.dve_spec import Src0, Src1, C0, C1
    return _register_snn(
        "SNN_STEP_ANT",
        lambda: (C0 >= Src0) + Src0 * C1 + Src1,
        lambda in0, in1, s0, s1, imm2: (
            (np.float32(s0) >= in0).astype(np.float32)
            + in0 * np.float32(s1) + in1
        ).astype(np.float32),
    )


def _register_snn_op2():
    # out = (s0 >= in0) + in0*s1 + in1*imm2
    from concourse.dve_spec import Src0, Src1, C0, C1, C2
    return _register_snn(
        "SNN_STEP2_ANT",
        lambda: (C0 >= Src0) + Src0 * C1 + Src1 * C2,
        lambda in0, in1, s0, s1, imm2: (
            (np.float32(s0) >= in0).astype(np.float32)
            + in0 * np.float32(s1) + in1 * np.float32(imm2)
        ).astype(np.float32),
    )


# ---------------- bass kernel trace ----------------
def trace_kernel(nc, xt_d, t_d, h_d, out_d):
    snn_op = _register_snn_op2()
    G = NPAIR // 4       # 16 stage-A groups of 4 chunks

    with TileContext(nc) as tc:
        with (
            tc.tile_pool(name="const", bufs=1) as cpool,
            tc.tile_pool(name="big", bufs=1) as bpool,
            tc.tile_pool(name="pa", bufs=2, space="PSUM") as pa_pool,
            tc.tile_pool(name="bq", bufs=2, space="PSUM") as bq_pool,
            tc.tile_pool(name="pt", bufs=2, space="PSUM") as pt_pool,
        ):
            # ---- constants + input, few big DMAs interleaved so stage A
            # can start after the first two issues ----
            ident = cpool.tile([H, H], F16)
            make_identity(nc, ident)
            dataT0 = bpool.tile([H, SP_FREE], F16, tag="dataT")
            tmats, hmats = {}, {}
            tmt = {}
            for layer in (0, 1):
                tm = cpool.tile([2 * T, 4 * T], F16, tag=f"t{layer}")
                tmt[layer] = tm
                tmats[layer] = (tm[:, :2 * T], tm[:, 2 * T:])
            nc.sync.dma_start(out=dataT0[:, 0:2048], in_=xt_d.ap()[:, 0:2048])
            nc.sync.dma_start(out=tmt[0], in_=t_d[0].ap())
            nc.sync.dma_start(out=dataT0[:, 2048:4096], in_=xt_d.ap()[:, 2048:4096])
            nc.sync.dma_start(out=dataT0[:, 4096:6144], in_=xt_d.ap()[:, 4096:6144])
            nc.sync.dma_start(out=dataT0[:, 6144:8192], in_=xt_d.ap()[:, 6144:8192])
            nc.sync.dma_start(out=tmt[1], in_=t_d[1].ap())
            for layer in (0, 1):
                hm = cpool.tile([H, 6 * H], F16, tag=f"h{layer}")
                nc.sync.dma_start(
                    out=hm[:, :].rearrange("p (s k n) -> p s k n", s=2, k=3),
                    in_=h_d[layer].ap().rearrange("s k p n -> p s k n"),
                )
                hmats[layer] = (hm[:, :3 * H], hm[:, 3 * H:])

            # padded mid pair; pad columns zeroed once (both layers reuse)
            midh = bpool.tile([H, MID_FREE], F16, tag="midh")
            midl = bpool.tile([H, MID_FREE], F16, tag="midl")
            for mtile in (midh, midl):
                for b in range(B_LOC):
                    nc.vector.memset(mtile[:, b * WP * T:b * WP * T + T], 0.0)
                    nc.vector.memset(
                        mtile[:, (b * WP + W + 1) * T:(b * WP + W + 2) * T], 0.0)

            dataT = dataT0
            for layer in (0, 1):
                d, theta_hat = _layer_consts(layer)
                two_d = float(np.float32(2.0 * d))
                md2 = float(np.float32(-(d * d)))
                thi, tlo = tmats[layer]
                hmh, hml = hmats[layer]

                # ---- stage A ----
                scopeA = nc.enter_named_scope(f"stageA{layer}", False)
                for g in range(G):
                    pa = pa_pool.tile([H, 4 * H], F32, tag="pa")
                    for c2 in range(4):
                        chunk = g * 4 + c2
                        lhsT = dataT[:, chunk * H:(chunk + 1) * H]
                        nc.tensor.matmul(
                            pa[:, c2 * H:(c2 + 1) * H], lhsT, thi,
                            start=True, stop=False, skip_group_check=True,
                        )
                        nc.tensor.matmul(
                            pa[:, c2 * H:(c2 + 1) * H], lhsT, tlo,
                            start=False, stop=True, skip_group_check=True,
                        )
                    b, w8 = divmod(g, G // B_LOC)
                    off = (b * WP + w8 * 8 + 1) * T
                    nc.scalar.copy(midh[:, off:off + 512], pa)
                    nc.vector.scalar_tensor_tensor(
                        midl[:, off:off + 512], midh[:, off:off + 512],
                        -1.0, pa, AO.mult, AO.add,
                    )
                nc.leave_named_scope(f"stageA{layer}", scopeA[0], False)

                # ---- stage B: f16 3-term, (b,w,t)-major blocks so the
                # moving operand's inner run is contiguous (16B bursts) ----
                what = bpool.tile([H, SP_FREE], F32, tag="what")
                whatv = what[:, :].rearrange("p (t b w) -> p t b w",
                                             t=T, b=B_LOC)
                mvh = midh[:, :].rearrange("p (b w t) -> p b w t", b=B_LOC, w=WP)
                mvl = midl[:, :].rearrange("p (b w t) -> p b w t", b=B_LOC, w=WP)
                scopeB = nc.enter_named_scope(f"stageB{layer}", False)
                for k in range(NBLK):
                    # one PSUM bank per b-half (2-bank tiles would round to 4)
                    bq0 = bq_pool.tile([H, BLK * W], F32, tag="bq0")
                    bq1 = bq_pool.tile([H, BLK * W], F32, tag="bq1")
                    bqs = (bq0, bq1)
                    ts = slice(k * BLK, (k + 1) * BLK)
                    first = True
                    for hm_, mv_ in ((hmh, mvh), (hmh, mvl), (hml, mvh)):
                        for dw in (0, -1, 1):
                            last = (hm_ is hml) and dw == 1
                            for b in range(B_LOC):
                                nc.tensor.matmul(
                                    bqs[b][:, :],
                                    hm_[:, (dw + 1) * H:(dw + 2) * H],
                                    mv_[:, b, 1 + dw:1 + dw + W, ts],
                                    start=first, stop=last,
                                    skip_group_check=True,
                                )
                            first = False
                    # strided evac: (w,t8) b-halves -> t-major what
                    for b in range(B_LOC):
                        src = bqs[b][:, :].rearrange("p (w t) -> p t w", t=BLK)
                        nc.scalar.copy(whatv[:, ts, b, :], src)
                nc.leave_named_scope(f"stageB{layer}", scopeB[0], False)

                # ---- scan: 1 fused DVE op/step + premerge on gpsimd ----
                mh = bpool.tile([H, SP_FREE], F32, tag="mh")
                spk = bpool.tile([H, SP_FREE], F16,
                                 tag="spk0" if layer == 0 else "spk1")
                spkv = spk[:, :].rearrange("p (b w t) -> p b w t",
                                           b=B_LOC, w=W)
                mhv = mh[:, :].rearrange("p (t b w) -> p b w t",
                                         t=T, b=B_LOC)

                def msl(t):
                    return mh[:, t * BW:(t + 1) * BW]

                def wsl(t):
                    return what[:, t * BW:(t + 1) * BW]

                prem = nc.gpsimd if PREMERGE == "gpsimd" else nc.vector

                # `what` holds Whatp = What/(-d^2) (H mats are host-scaled);
                # the DVE op multiplies in1 by imm2 = -d^2, so the premerge
                # is a PLAIN ADD (the only 2-tensor op GpSimd's ISA has):
                #   P[t+1]  = Whatp[t+1] + m[t-1]          (gpsimd, in place)
                #   m[t+1]  = (th>=m[t]) + 2d*m[t] + md2*P[t+1]   (DVE)
                scopeS = nc.enter_named_scope(f"scan{layer}", False)
                nc.vector.tensor_scalar(msl(0), wsl(0), md2, None, AO.mult)
                def vslot(s):
                    # slots premerged inline on vector; chosen so gpsimd is
                    # idle while the (SBUF-port-sharing) vector threshold
                    # slab runs right after steps 7/15/23/...
                    return s % 8 in (1, 2, 5)

                for t in range(T - 1):
                    # vector premerge for slot t+2, emitted BEFORE this
                    # step's DVE op so its drain is hidden by one op gap
                    s = t + 2
                    if 2 <= s <= T - 1 and vslot(s):
                        nc.vector.tensor_tensor(
                            wsl(s), msl(s - 2), wsl(s), AO.add)
                    # gpsimd premerge for slot t+1 (2-step slack)
                    s = t + 1
                    if s >= 2 and not vslot(s):
                        prem.tensor_tensor(
                            wsl(s), msl(s - 2), wsl(s), AO.add)
                    nc.vector._custom_dve(
                        snn_op, out=msl(t + 1), in0=msl(t),
                        in1=wsl(t + 1), s0=theta_hat, s1=two_d, imm2=md2,
                    )
                    # early final piece so only 2 steps remain after the loop
                    if layer == 1 and t == T - 3:
                        sl = slice((T - 8) * BW, (T - 2) * BW)
                        nc.vector.tensor_scalar(
                            spk[:, sl], mh[:, sl], theta_hat, None,
                            AO.is_le)
                        nc.sync.dma_start(
                            out=out_d.ap()[:, sl], in_=spk[:, sl])
                    # keep the PE HAM warm through scan0's tail (B0 is done
                    # by then) so trans1+A1 start at 2.4GHz, not 1.2
                    if layer == 0 and t >= 24 and t % 4 == 1:
                        pw = pt_pool.tile([H, 4 * H], F16, tag="ptr")
                        nc.tensor.transpose(
                            pw[:, :H], msl(t).bitcast(F16)[:, :H], ident)
                    # threshold finished 8-step slabs (small pieces so the
                    # vector queue never blocks for long)
                    if layer == 0 and (t + 1) % 4 == 0 and (t + 1) < T:
                        t0s = t + 1 - 4
                        nc.vector.tensor_scalar(
                            spkv[:, :, :, t0s:t + 1],
                            mhv[:, :, :, t0s:t + 1],
                            theta_hat, None, AO.is_le)
                    if (t + 1) % 8 == 0 and (t + 1) < T:
                        t0s = t + 1 - 8
                        if layer == 0:
                            pass
                        else:
                            sl = slice(t0s * BW, (t + 1) * BW)
                            nc.vector.tensor_scalar(
                                spk[:, sl], mh[:, sl], theta_hat, None,
                                AO.is_le)
                            nc.sync.dma_start(
                                out=out_d.ap()[:, sl], in_=spk[:, sl])
                nc.leave_named_scope(f"scan{layer}", scopeS[0], False)
                if layer == 0:
                    nc.vector.tensor_scalar(
                        spkv[:, :, :, T - 4:T], mhv[:, :, :, T - 4:T],
                        theta_hat, None, AO.is_le)
                else:
                    sl = slice((T - 2) * BW, T * BW)
                    nc.vector.tensor_scalar(
                        spk[:, sl], mh[:, sl], theta_hat, None, AO.is_le)
                    nc.sync.dma_start(out=out_d.ap()[:, sl], in_=spk[:, sl])

                if layer == 0:
                    # transpose s1 chunks on PE -> next layer's dataT
                    dataT2 = bpool.tile([H, SP_FREE], F16, tag="dataT2")
                    scopeT = nc.enter_named_scope("trans1", False)
                    for g in range(G):
                        ptr = pt_pool.tile([H, 4 * H], F16, tag="ptr")
                        for c2 in range(4):
                            chunk = g * 4 + c2
                            nc.tensor.transpose(
                                ptr[:, c2 * H:(c2 + 1) * H],
                                spk[:, chunk * H:(chunk + 1) * H], ident)
                        sl = slice(g * 512, (g + 1) * 512)
                        if g % 2 == 0:
                            nc.scalar.copy(dataT2[:, sl], ptr)
                        else:
                            nc.vector.tensor_copy(dataT2[:, sl], ptr)
                    nc.leave_named_scope("trans1", scopeT[0], False)
                    dataT = dataT2
    return nc


_BUILT = {}


def _build():
    global _BUILT
    key = (PREMERGE, THR_ENGINE)
    if key in _BUILT:
        return _BUILT[key]
    nc = bacc.Bacc("TRN2", debug=False)
    xt_d = nc.dram_tensor("xt", [H, SP_FREE], F16, kind="ExternalInput")
    t_d, h_d = {}, {}
    for layer in (0, 1):
        t_d[layer] = nc.dram_tensor(f"t{layer}", [2 * T, 4 * T], F16,
                                    kind="ExternalInput")
        h_d[layer] = nc.dram_tensor(f"h{layer}", [2, 3, H, H], F16,
                                    kind="ExternalInput")
    out_d = nc.dram_tensor("out", [H, SP_FREE], F16, kind="ExternalOutput")
    trace_kernel(nc, xt_d, t_d, h_d, out_d)
    nc.compile()
    _BUILT[key] = nc
    return nc


def _host_inputs(conv1_w, conv2_w):
    ins = {}
    for layer, w in ((0, conv1_w), (1, conv2_w)):
        hi, lo = _hilo_f16_blockdiag(_temporal_mat(layer))
        ins[f"t{layer}"] = np.hstack([hi, lo])
        d, _ = _layer_consts(layer)
        md2 = float(np.float32(-(d * d)))
        # stage B computes Whatp = What/(-d^2); the scan's DVE op multiplies
        # the premerged stream back by -d^2 (imm2)
        w2 = np.asarray(w, np.float32).reshape(3, 3) / np.float32(md2)
        ins[f"h{layer}"] = np.stack([_h_mats(w2, "hi"), _h_mats(w2, "lo")])
    return ins


def make_in_maps(spikeInput, conv1_w, conv2_w):
    x = np.asarray(spikeInput, np.float32).reshape(B_FULL, H, W, T)
    x16 = x.astype(np.float16)                      # exact: values are 0/1
    common = _host_inputs(conv1_w, conv2_w)
    in_maps = []
    for c in range(N_CORES):
        xc = x16[c * B_LOC:(c + 1) * B_LOC]         # [b, h, w, t]
        xc = xc.reshape(B_LOC, H, W // 2, 2, T)     # b h wp w2 t
        xt = np.ascontiguousarray(xc.transpose(3, 4, 0, 2, 1))  # w2 t b wp h
        m = dict(common)
        m["xt"] = xt.reshape(H, SP_FREE)
        in_maps.append(m)
    return in_maps


def kernel(spikeInput, conv1_w, conv2_w):
    nc = _build()
    in_maps = make_in_maps(spikeInput, conv1_w, conv2_w)
    res = bass_utils.run_bass_kernel_spmd(nc, in_maps, core_ids=list(range(N_CORES)))
    outs = []
    for r in res.results:
        o = r["out"].reshape(H, T, B_LOC, W)        # h t b w
        outs.append(o.transpose(2, 0, 3, 1))        # b h w t
    return np.concatenate(outs, axis=0).astype(np.float32)
